# revision 1
# baseline (speedup 1.0000x reference)
"""AgentAttention Trainium2 kernel.

Sharding: data-parallel over batch B=16 across 8 NeuronCores (2 items/core),
no collectives. Per batch item (C=256, N=56*56=3136, 8 heads, hd=32, 49
agents), everything in bf16 matmuls with fp32 PSUM accumulation:

  qkvT[768,N]  = qkv_w @ x                       (PE, feature-major)
  a[C,49]      = 8x8 avg-pool of x               (DVE reduces)
  S1T[N,(h,49)]= kT-chunk.T @ agentT-blockdiag   (PE, 4 heads per pass)
  expS1T       = exp(scale*S1T)                  (ACT; scores are O(0.3) so
                                                  softmax needs no max-subtract)
  agent_v      = expS1T.T @ [v_nm | 1]           (PE; ones-col = denominator)
  w2T[49,N]/h  = exp(scale * agBD2.T @ qT)       (PE+ACT, 2 heads per pass)
  s2           = onesT @ w2T                     (PE; stage-2 denominators)
  u[C,N]       = av_blockdiag.T @ w2T            (PE)
  x_att        = u * bcast(1/s2)                 (DRAM-bounce row-bcast + DVE)
  dwc          = 3x3 depthwise conv: 7 taps as accumulating diag matmuls on
                 PE, 1 tap each on DVE/GPSIMD as per-partition-scalar FMA
  out          = proj_w @ (x_att + dwc + b) + proj_b + x    (PE + DVE)

The stage-2 tail (denominators -> DRAM-bounce broadcast -> reciprocal ->
combine) is pipelined per pair-group: group a's chain runs while group
(a+1)'s scores/exp are still on PE/ACT, and combine for channel-tile g==a
starts as soon as its rbc slice is ready.
"""

import sys

for _p in ("/opt/trn_rl_repo", "/opt/trn_rl_repo/concourse"):
    if _p not in sys.path:
        sys.path.insert(0, _p)

import numpy as np
import ml_dtypes

import concourse.bass as bass
import concourse.bacc as bacc
import concourse.mybir as mybir
import concourse.tile as tile
from concourse.bass_utils import run_bass_kernel_spmd

BF16 = ml_dtypes.bfloat16
FP32 = np.float32

B, C, HH, WW = 16, 256, 56, 56
N = HH * WW            # 3136
NH, HD, A = 8, 32, 49
SCALE = float(HD) ** -0.5
NCORES = 8
BPC = B // NCORES      # 2
CT = 2                 # 128-channel tiles
NP = 25                # ceil(N/128); last chunk is 64
FCH = 448              # free-dim chunk = 8 image rows
NF = 7
PAD = 58
BD = mybir.dt.bfloat16
FD = mybir.dt.float32
FX = mybir.ActivationFunctionType
OP = mybir.AluOpType

TAPS = [(dy, dx) for dy in range(3) for dx in range(3)]
PE_TAPS = [0, 1, 2, 4, 6, 7, 8]  # diag matmuls on the tensor engine
DVE_TAP = 3                      # FMA on DVE (chained after the division)
GP_TAP = 5                       # FMA on gpsimd


def _pchunk(i):
    n0 = 128 * i
    return n0, min(128, N - n0)


def _grp(n, size):
    return [list(range(s, min(s + size, n))) for s in range(0, n, size)]


def build_bass(reps=1):
    nc = bacc.Bacc()
    d = {}
    d["xbf"] = nc.declare_dram_parameter("xbf", [BPC, C, N], BD, isOutput=False)
    d["x32"] = nc.declare_dram_parameter("x32", [BPC, C, N], FD, isOutput=False)
    d["wqkvT"] = nc.declare_dram_parameter("wqkvT", [C, 768], BD, isOutput=False)
    d["wprojT"] = nc.declare_dram_parameter("wprojT", [C, C], BD, isOutput=False)
    d["projb"] = nc.declare_dram_parameter("projb", [C, 1], FD, isOutput=False)
    d["dwcdiag"] = nc.declare_dram_parameter("dwcdiag",
                                             [len(PE_TAPS), CT, 128, 128], BD,
                                             isOutput=False)
    d["dwcw"] = nc.declare_dram_parameter("dwcw", [C, 9], FD, isOutput=False)
    d["dwcb"] = nc.declare_dram_parameter("dwcb", [C, 1], FD, isOutput=False)
    d["out32"] = nc.declare_dram_parameter("out32", [BPC, C, N], FD,
                                           isOutput=True)
    # internal DRAM scratch: bounce for the s2 row-broadcast (SBUF-source
    # DMAs cannot have partition step 0; DRAM-source DMAs can)
    d["s2d"] = nc.dram_tensor("s2d", [NH, N], BD)
    with tile.TileContext(nc) as tc:
        _emit(nc, tc, d, reps)
    nc.finalize()
    return nc


def _emit(nc, tc, d, reps=1):
    import contextlib
    ctx = contextlib.ExitStack()
    with ctx:
        persist = ctx.enter_context(tc.tile_pool(name="persist", bufs=1))
        small = ctx.enter_context(tc.tile_pool(name="small", bufs=2))
        upch = ctx.enter_context(tc.tile_pool(name="upch", bufs=3))
        x32p = ctx.enter_context(tc.tile_pool(name="x32p", bufs=2))
        # PSUM: "big" 2-bank tiles x2 + "pk" 1-bank tiles x4 = 8 banks
        psum = ctx.enter_context(tc.tile_pool(name="psum", bufs=1,
                                              space="PSUM"))

        t = {}
        t["wqkv"] = persist.tile([128, 2, 768], BD, name="wqkv")
        t["wproj"] = persist.tile([128, 2, C], BD, name="wproj")
        t["projb"] = persist.tile([128, 2, 1], FD, name="projb")
        t["dwcb"] = persist.tile([128, 2, 1], FD, name="dwcb")
        t["dwcw"] = persist.tile([128, 2, 9], FD, name="dwcw")
        t["dwcdiag"] = persist.tile([128, len(PE_TAPS), CT, 128], BD,
                                    name="dwcdiag")
        t["xbf"] = persist.tile([128, CT, N], BD, name="xbf")
        t["qk"] = persist.tile([128, 4, N], BD, name="qk")
        t["vpad"] = persist.tile([128, CT, PAD * PAD], BD, name="vpad")
        t["v_nm"] = persist.tile([128, NP, NH, 33], BD, name="v_nm")
        t["es1"] = persist.tile([128, NP, NH, 64], BD, name="es1")
        t["w2T"] = persist.tile([128, 4, N], BD, name="w2T")
        t["aT"] = persist.tile([128, CT, A], FD, name="aT")
        t["agBD1"] = persist.tile([128, CT, 4 * A], BD, name="agBD1")
        t["agBD2"] = persist.tile([128, CT, 113], BD, name="agBD2")
        t["onesBD"] = persist.tile([128, 2], BD, name="onesBD")
        t["av_ext"] = persist.tile([128, 4, 64], BD, name="av_ext")
        t["rtile"] = persist.tile([128, 8], FD, name="rtile")
        # s2raw: [row 64*(p%2)+e, block p//2, n]
        t["s2raw"] = persist.tile([128, 2, N], BD, name="s2raw")
        t["rbc"] = persist.tile([128, CT, N], BD, name="rbc")
        t["sum"] = persist.tile([128, CT, N], BD, name="sum")

        # --- one-time init: only the zero/ones regions later matmuls read
        # but no per-item pass rewrites (they persist across batch items)
        nc.vector.memset(t["es1"][:, :, :, A:64], 0.0)       # head pads
        nc.vector.memset(t["v_nm"][:, :, :, 32:33], 1.0)     # ones cols
        nc.vector.memset(t["vpad"][:, :, 0:PAD], 0.0)        # top border
        nc.vector.memset(t["vpad"][:, :, 57 * PAD:58 * PAD], 0.0)
        nc.vector.memset(
            t["vpad"][:, :, :].rearrange("p g (r c) -> p g r c", c=PAD)
            [:, :, 1:57, 0:1], 0.0)                          # left border
        nc.vector.memset(
            t["vpad"][:, :, :].rearrange("p g (r c) -> p g r c", c=PAD)
            [:, :, 1:57, 57:58], 0.0)                        # right border
        nc.vector.memset(t["agBD1"][:, :, :], 0.0)
        nc.vector.memset(t["agBD2"][:, :, :], 0.0)
        nc.vector.memset(t["av_ext"][:, :, :], 0.0)
        nc.vector.memset(t["onesBD"][:, :], 0.0)
        nc.vector.memset(t["onesBD"][0:A, 0:1], 1.0)
        nc.vector.memset(t["onesBD"][64:113, 1:2], 1.0)
        # chunk-24 garbage rows of v_nm/es1 are never read (K=64 there)

        nc.sync.dma_start(out=t["wqkv"][:, :, :],
                          in_=d["wqkvT"].rearrange("(t p) f -> p t f", p=128))
        nc.sync.dma_start(out=t["wproj"][:, :, :],
                          in_=d["wprojT"].rearrange("(t p) f -> p t f", p=128))
        nc.sync.dma_start(out=t["projb"][:, :, :],
                          in_=d["projb"].rearrange("(t p) o -> p t o", p=128))
        nc.sync.dma_start(out=t["dwcb"][:, :, :],
                          in_=d["dwcb"].rearrange("(t p) o -> p t o", p=128))
        nc.sync.dma_start(out=t["dwcw"][:, :, :],
                          in_=d["dwcw"].rearrange("(t p) o -> p t o", p=128))
        nc.sync.dma_start(out=t["dwcdiag"][:, :, :, :],
                          in_=d["dwcdiag"].rearrange("t g p f -> p t g f"))

        pools = dict(psum=psum, small=small, upch=upch, x32p=x32p)
        for _ in range(reps):
            for b in range(BPC):
                _emit_item(nc, tc, b, d, t, pools)


def _big(pools, nm):
    return pools["psum"].tile([128, 1024], FD, name=nm, tag="pbig", bufs=2)


def _bank(pools, nm):
    return pools["psum"].tile([128, 512], FD, name=nm, tag="pk", bufs=4)


def _emit_item(nc, tc, b, d, t, pools):
    # ---------------- load x ----------------
    for g in range(CT):
        nc.sync.dma_start(out=t["xbf"][:, g, :],
                          in_=d["xbf"][b, 128 * g:128 * (g + 1), :])
    x32t = pools["x32p"].tile([128, CT, N], FD, name="x32t", tag="x32t",
                              bufs=1)
    nc.sync.dma_start(out=x32t[:, :, :],
                      in_=d["x32"][b].rearrange("(g p) n -> p g n", p=128))

    # ---------------- qkv projection ----------------
    # qkvT[f, n] = sum_c wqkvT[c, f] x[c, n]; mt 0..3 -> qk, 4..5 -> vpad
    for mt in range(6):
        for js in _grp(NF, 2):
            pt = _big(pools, "pq")
            j0, nj = js[0], len(js)
            for j in js:
                for kt in range(CT):
                    nc.tensor.matmul(
                        pt[:, 512 * (j - j0):512 * (j - j0) + FCH],
                        t["wqkv"][:, kt, 128 * mt:128 * (mt + 1)],
                        t["xbf"][:, kt, FCH * j:FCH * (j + 1)],
                        start=(kt == 0), stop=(kt == CT - 1))
            src = (pt[:, 0:512 * nj].rearrange("p (j f) -> p j f", f=512)
                   [:, :, 0:FCH])
            if mt < 4:
                dst = (t["qk"][:, mt, :].rearrange("p (j f) -> p j f", f=FCH)
                       [:, j0:j0 + nj, :])
                if (j0 // 2) % 2 == 0:
                    nc.vector.tensor_copy(out=dst, in_=src)
                else:
                    nc.scalar.copy(out=dst, in_=src)
            else:
                g = mt - 4
                rows = (t["vpad"][:, g, :]
                        .rearrange("p (r c) -> p r c", c=PAD)
                        [:, 1 + 8 * j0:1 + 8 * j0 + 8 * nj, 1:57]
                        .rearrange("p (j r) c -> p j r c", r=8))
                nc.vector.tensor_copy(
                    out=rows, in_=src.rearrange("p j (r c) -> p j r c", c=56))

    # ---------------- pooling -> agent blocks ----------------
    for g in range(CT):
        a1 = pools["small"].tile([128, 56 * 7], FD, name="a1", tag="a1")
        nc.vector.tensor_reduce(
            out=a1[:, :],
            in_=t["xbf"][:, g, :].rearrange("p (h wb wi) -> p h wb wi", wb=7,
                                            wi=8),
            axis=mybir.AxisListType.X, op=OP.add)
        nc.vector.tensor_reduce(
            out=t["aT"][:, g, :],
            in_=a1[:, :].rearrange("p (hb hi wb) -> p hb wb hi", hb=7, hi=8,
                                   wb=7),
            axis=mybir.AxisListType.X, op=OP.add)
        # the two 8-sums fold into one /64; scale applied during the copies
        for hp in range(4):
            nc.vector.tensor_scalar_mul(
                out=t["agBD1"][32 * hp:32 * (hp + 1), g, A * hp:A * (hp + 1)],
                in0=t["aT"][32 * hp:32 * (hp + 1), g, :], scalar1=1.0 / 64.0)
        for q in range(2):
            nc.vector.tensor_scalar_mul(
                out=t["agBD2"][64 * q:64 * q + 32, g, 0:A],
                in0=t["aT"][64 * q:64 * q + 32, g, :], scalar1=1.0 / 64.0)
            nc.vector.tensor_scalar_mul(
                out=t["agBD2"][64 * q + 32:64 * q + 64, g, 64:113],
                in0=t["aT"][64 * q + 32:64 * q + 64, g, :], scalar1=1.0 / 64.0)

    # ---- stage-1 scores/exp + v_nm + agent_v, interleaved per chunk-group
    # (keeps PE on v_nm/agv matmuls while ACT runs the stage-1 exps, and DVE
    # on v_nm copies; the agent_v accumulation trails one group behind)
    agv = _bank(pools, "agv")
    agv3 = agv[:, 0:264].rearrange("p (q f) -> p q f", f=66)

    def s1_group(g, iis):
        pt = _big(pools, "p1")
        i0 = iis[0]
        for i in iis:
            n0, sz = _pchunk(i)
            nc.tensor.matmul(
                pt[0:sz, 512 * (i - i0):512 * (i - i0) + 196],
                t["qk"][:, 2 + g, n0:n0 + sz],
                t["agBD1"][:, g, :], start=True, stop=True)
        full = [i for i in iis if _pchunk(i)[1] == 128]
        if full:
            nfull = len(full)
            src = (pt[:, 512 * (full[0] - i0):
                      512 * (full[0] - i0) + 512 * nfull]
                   .rearrange("p (j f) -> p j f", f=512)[:, :, 0:196]
                   .rearrange("p j (h a) -> p j h a", a=A))
            nc.scalar.activation(
                out=t["es1"][:, full[0]:full[0] + nfull, 4 * g:4 * g + 4, 0:A],
                in_=src, func=FX.Exp, scale=SCALE)
        for i in iis:
            n0, sz = _pchunk(i)
            if sz == 128:
                continue
            src = (pt[0:sz, 512 * (i - i0):512 * (i - i0) + 196]
                   .rearrange("p (h a) -> p h a", a=A))
            nc.scalar.activation(
                out=t["es1"][0:sz, i, 4 * g:4 * g + 4, 0:A],
                in_=src, func=FX.Exp, scale=SCALE)

    def v_group(iis):
        pt = _big(pools, "pv")
        i0, nj = iis[0], len(iis)
        for i in iis:
            n0, sz = _pchunk(i)
            for kt in range(CT):
                nc.tensor.matmul(
                    pt[0:sz, 512 * (i - i0):512 * (i - i0) + C],
                    t["xbf"][:, kt, n0:n0 + sz],
                    t["wqkv"][:, kt, 512:768],
                    start=(kt == 0), stop=(kt == CT - 1))
        src = (pt[:, 0:512 * nj].rearrange("p (j f) -> p j f", f=512)
               [:, :, 0:C].rearrange("p j (h dd) -> p j h dd", dd=32))
        nc.vector.tensor_copy(out=t["v_nm"][:, i0:i0 + nj, :, 0:32], in_=src)

    def agv_group(iis):
        for i in iis:
            n0, sz = _pchunk(i)
            for p in range(4):
                nc.tensor.matmul(
                    agv3[:, p, :],
                    t["es1"][0:sz, i, 2 * p:2 * p + 2, :].rearrange(
                        "p h dd -> p (h dd)"),
                    t["v_nm"][0:sz, i, 2 * p:2 * p + 2, :].rearrange(
                        "p h dd -> p (h dd)"),
                    start=(i == 0), stop=(i == NP - 1))

    groups = _grp(NP, 2)
    for k, iis in enumerate(groups):
        s1_group(0, iis)
        s1_group(1, iis)
        v_group(iis)
        if k > 0:
            agv_group(groups[k - 1])
    agv_group(groups[-1])

    nc.vector.reciprocal(out=t["rtile"][0:A, 0:4], in_=agv3[0:A, :, 32])
    nc.vector.reciprocal(out=t["rtile"][64:113, 4:8], in_=agv3[64:113, :, 65])
    with nc.allow_low_precision("bf16 attention weights"):
        nc.vector.tensor_tensor(
            out=t["av_ext"][0:A, :, 0:32], in0=agv3[0:A, :, 0:32],
            in1=t["rtile"][0:A, 0:4].unsqueeze(2).to_broadcast([A, 4, 32]),
            op=OP.mult)
        nc.vector.tensor_tensor(
            out=t["av_ext"][64:113, :, 32:64], in0=agv3[64:113, :, 33:65],
            in1=t["rtile"][64:113, 4:8].unsqueeze(2).to_broadcast([A, 4, 32]),
            op=OP.mult)

    # ------- stage 2: score/exp + denominator chains for both groups -----
    for a in range(2):
        # scores + exp for pairs 2a (rows 0:64) and 2a+1 (rows 64:128);
        # interleaved so the PE overlaps them on disjoint row groups
        for js in _grp(NF, 2):
            j0, nj = js[0], len(js)
            pts = [_big(pools, "p2"), _big(pools, "p2")]
            for j in js:
                for q in range(2):
                    p = 2 * a + q
                    nc.tensor.matmul(
                        pts[q][0:113, 512 * (j - j0):512 * (j - j0) + FCH],
                        t["agBD2"][64 * q:64 * (q + 1), a, :],
                        t["qk"][64 * q:64 * (q + 1), a,
                                FCH * j:FCH * (j + 1)],
                        start=True, stop=True)
            for q in range(2):
                p = 2 * a + q
                src = (pts[q][0:113, 0:512 * nj]
                       .rearrange("p (j f) -> p j f", f=512)[:, :, 0:FCH])
                dst = (t["w2T"][0:113, p, :]
                       .rearrange("p (j f) -> p j f", f=FCH)
                       [:, j0:j0 + nj, :])
                nc.scalar.activation(out=dst, in_=src, func=FX.Exp,
                                     scale=SCALE)
        # denominators for this pair group
        for js in _grp(NF, 2):
            j0, nj = js[0], len(js)
            pt = _big(pools, "po")
            for j in js:
                for q in range(2):
                    nc.tensor.matmul(
                        pt[64 * q:64 * q + 2,
                           512 * (j - j0):512 * (j - j0) + FCH],
                        t["onesBD"][0:113, :],
                        t["w2T"][0:113, 2 * a + q, FCH * j:FCH * (j + 1)],
                        start=True, stop=True)
            pt_ap = pt[:, :]
            s2_ap = t["s2raw"][:, :, :]
            with nc.allow_low_precision("bf16 softmax denominators"):
                for q in range(2):
                    src = bass.AP(tensor=pt_ap.tensor,
                                  offset=pt_ap.offset + (64 * q) * 1024,
                                  ap=[[1024, 2], [512, nj], [1, FCH]])
                    dst = bass.AP(tensor=s2_ap.tensor,
                                  offset=(s2_ap.offset + (64 * q) * (2 * N)
                                          + a * N + FCH * j0),
                                  ap=[[2 * N, 2], [FCH, nj], [1, FCH]])
                    nc.vector.reciprocal(out=dst, in_=src)
        # broadcast 1/s2 to the 32 channel rows of each head (DRAM bounce;
        # heads of group a are exactly channel-tile g = a)
        for h in range(4 * a, 4 * a + 4):
            p, e = h // 2, h % 2
            q = p % 2
            src = t["s2raw"][64 * q + e:64 * q + e + 1, a, :]
            dst = bass.AP(tensor=d["s2d"], offset=h * N, ap=[[N, 1], [1, N]])
            nc.scalar.dma_start(out=dst, in_=src)
        for h in range(4 * a, 4 * a + 4):
            src = bass.AP(tensor=d["s2d"], offset=h * N, ap=[[0, 32], [1, N]])
            nc.scalar.dma_start(out=t["rbc"][32 * (h % 4):32 * (h % 4) + 32,
                                             a, :], in_=src)


    # ---------------- dwc + apply + combine ----------------
    for g in range(2):
        vp = t["vpad"][:, g, :].rearrange("p (r c) -> p r c", c=PAD)
        for j in range(NF):
            pd = _bank(pools, "pd")
            pa = _bank(pools, "pa")
            # PE taps (diag matmuls accumulate) + apply (row-split pairs)
            for k, ti in enumerate(PE_TAPS):
                dy, dx = TAPS[ti]
                nc.tensor.matmul(
                    pd[:, 0:FCH].rearrange("p (r c) -> p r c", c=56),
                    t["dwcdiag"][:, k, g, :],
                    vp[:, 8 * j + dy:8 * j + dy + 8, dx:dx + 56],
                    start=(k == 0), stop=(k == len(PE_TAPS) - 1))
            for q in range(2):
                nc.tensor.matmul(
                    pa[64 * q:64 * (q + 1), 0:FCH],
                    t["av_ext"][0:113, 2 * g + q, :],
                    t["w2T"][0:113, 2 * g + q, FCH * j:FCH * (j + 1)],
                    start=True, stop=True)
            upt = pools["upch"].tile([128, FCH], BD, name="up", tag="up")
            gac = pools["upch"].tile([128, FCH], BD, name="gac", tag="gac")
            with nc.allow_low_precision("bf16 activations"):
                # u' = u * (1/s2)          (DVE, PSUM read)
                nc.vector.tensor_tensor(
                    out=upt[:, :], in0=pa[:, 0:FCH],
                    in1=t["rbc"][:, g, FCH * j:FCH * (j + 1)], op=OP.mult)
                # u' += dve-tap            (DVE, SBUF FMA)
                dy, dx = TAPS[DVE_TAP]
                nc.vector.scalar_tensor_tensor(
                    out=upt[:, :].rearrange("p (r c) -> p r c", c=56),
                    in0=vp[:, 8 * j + dy:8 * j + dy + 8, dx:dx + 56],
                    scalar=t["dwcw"][:, g, DVE_TAP:DVE_TAP + 1],
                    in1=upt[:, :].rearrange("p (r c) -> p r c", c=56),
                    op0=OP.mult, op1=OP.add)
                # gac = gp-tap; gac += u'  (gpsimd, SBUF)
                dy, dx = TAPS[GP_TAP]
                nc.gpsimd.tensor_scalar_mul(
                    out=gac[:, :].rearrange("p (r c) -> p r c", c=56),
                    in0=vp[:, 8 * j + dy:8 * j + dy + 8, dx:dx + 56],
                    scalar1=t["dwcw"][:, g, GP_TAP:GP_TAP + 1])
                nc.gpsimd.tensor_tensor(out=gac[:, :], in0=gac[:, :],
                                        in1=upt[:, :], op=OP.add)
                # sum = (dwc_pe + dwc_b) + gac   (DVE, PSUM read)
                nc.vector.scalar_tensor_tensor(
                    out=t["sum"][:, g, FCH * j:FCH * (j + 1)],
                    in0=pd[:, 0:FCH], scalar=t["dwcb"][:, g, :],
                    in1=gac[:, :], op0=OP.add, op1=OP.add)

    # ---------------- proj + bias + residual ----------------
    for mt in range(CT):
        for js in _grp(NF, 2):
            j0, nj = js[0], len(js)
            pp = _big(pools, "pp")
            for j in js:
                for kt in range(CT):
                    nc.tensor.matmul(
                        pp[:, 512 * (j - j0):512 * (j - j0) + FCH],
                        t["wproj"][:, kt, 128 * mt:128 * (mt + 1)],
                        t["sum"][:, kt, FCH * j:FCH * (j + 1)],
                        start=(kt == 0), stop=(kt == CT - 1))
            src = (pp[:, 0:512 * nj].rearrange("p (j f) -> p j f", f=512)
                   [:, :, 0:FCH])
            dstx = (x32t[:, mt, :].rearrange("p (j f) -> p j f", f=FCH)
                    [:, j0:j0 + nj, :])
            nc.vector.scalar_tensor_tensor(
                out=dstx, in0=src, scalar=t["projb"][:, mt, :], in1=dstx,
                op0=OP.add, op1=OP.add)
        nc.gpsimd.dma_start(out=d["out32"][b, 128 * mt:128 * (mt + 1), :],
                            in_=x32t[:, mt, :])


def host_prep(x, qkv_w, proj_w, proj_b, dwc_w, dwc_b):
    xf = np.ascontiguousarray(x.reshape(B, C, N), dtype=FP32)
    wqkvT = np.ascontiguousarray(np.asarray(qkv_w, FP32).T.astype(BF16))
    wprojT = np.ascontiguousarray(np.asarray(proj_w, FP32).T.astype(BF16))
    projb = np.ascontiguousarray(np.asarray(proj_b, FP32).reshape(C, 1))
    dwcb = np.ascontiguousarray(np.asarray(dwc_b, FP32).reshape(C, 1))
    w33 = np.asarray(dwc_w, FP32).reshape(C, 9)
    dd = np.zeros((len(PE_TAPS), CT, 128, 128), FP32)
    for k, ti in enumerate(PE_TAPS):
        for g in range(CT):
            np.fill_diagonal(dd[k, g], w33[128 * g:128 * (g + 1), ti])
    dwcdiag = np.ascontiguousarray(dd.astype(BF16))
    dwcw = np.ascontiguousarray(w33)
    maps = []
    for c in range(NCORES):
        xs = xf[BPC * c:BPC * (c + 1)]
        maps.append(dict(
            xbf=np.ascontiguousarray(xs.astype(BF16)),
            x32=np.ascontiguousarray(xs),
            wqkvT=wqkvT, wprojT=wprojT, projb=projb,
            dwcdiag=dwcdiag, dwcw=dwcw, dwcb=dwcb,
        ))
    return maps


_NC_CACHE = {}


def kernel(x, qkv_w, proj_w, proj_b, dwc_w, dwc_b, trace=False):
    if "nc" not in _NC_CACHE:
        _NC_CACHE["nc"] = build_bass()
    nc = _NC_CACHE["nc"]
    maps = host_prep(np.asarray(x), np.asarray(qkv_w), np.asarray(proj_w),
                     np.asarray(proj_b), np.asarray(dwc_w), np.asarray(dwc_b))
    res = run_bass_kernel_spmd(nc, maps, core_ids=list(range(NCORES)),
                               trace=trace)
    outs = [r["out32"].reshape(BPC, C, HH, WW) for r in res.results]
    full = np.concatenate(outs, axis=0).astype(np.float32)
    if trace:
        return full, res
    return full



# revision 17
# speedup vs baseline: 1.6295x; 1.6295x over previous
"""AgentAttention Trainium2 kernel.

Sharding: data-parallel over batch B=16 across 8 NeuronCores (2 items/core),
no collectives. Per batch item (C=256, N=56*56=3136, 8 heads, hd=32, 49
agents), everything in bf16 matmuls with fp32 PSUM accumulation:

  qkvT[768,N]  = qkv_w @ x                       (PE, feature-major)
  a[C,49]      = 8x8 avg-pool of x               (DVE reduces)
  S1T[N,(h,49)]= kT-chunk.T @ agentT-blockdiag   (PE, 4 heads per pass)
  expS1T       = exp(scale*S1T)                  (ACT; scores are O(0.3) so
                                                  softmax needs no max-subtract)
  agent_v      = expS1T.T @ [v_nm | 1]           (PE; ones-col = denominator)
  w2T[49,N]/h  = exp(scale * agBD2.T @ qT)       (PE+ACT, 2 heads per pass)
  s2           = onesT @ w2T                     (PE; stage-2 denominators)
  u[C,N]       = av_blockdiag.T @ w2T            (PE)
  x_att        = u * bcast(1/s2)                 (DRAM-bounce row-bcast + DVE)
  dwc          = 3x3 depthwise conv: 7 taps as accumulating diag matmuls on
                 PE, 1 tap each on DVE/GPSIMD as per-partition-scalar FMA
  out          = proj_w @ (x_att + dwc + b) + proj_b + x    (PE + DVE)

The stage-2 tail (denominators -> DRAM-bounce broadcast -> reciprocal ->
combine) is pipelined per pair-group: group a's chain runs while group
(a+1)'s scores/exp are still on PE/ACT, and combine for channel-tile g==a
starts as soon as its rbc slice is ready.
"""

import sys

for _p in ("/opt/trn_rl_repo", "/opt/trn_rl_repo/concourse"):
    if _p not in sys.path:
        sys.path.insert(0, _p)

import numpy as np
import ml_dtypes

import concourse.bass as bass
import concourse.bacc as bacc
import concourse.mybir as mybir
import concourse.tile as tile
from concourse.bass_utils import run_bass_kernel_spmd

BF16 = ml_dtypes.bfloat16
FP32 = np.float32

B, C, HH, WW = 16, 256, 56, 56
N = HH * WW            # 3136
NH, HD, A = 8, 32, 49
SCALE = float(HD) ** -0.5
NCORES = 8
BPC = B // NCORES      # 2
CT = 2                 # 128-channel tiles
NP = 25                # ceil(N/128); last chunk is 64
FCH = 448              # free-dim chunk = 8 image rows
NF = 7
PAD = 58
BD = mybir.dt.bfloat16
FD = mybir.dt.float32
FX = mybir.ActivationFunctionType
OP = mybir.AluOpType

TAPS = [(dy, dx) for dy in range(3) for dx in range(3)]
PE_TAPS = [0, 1, 2, 4, 5, 6, 7, 8]  # diag matmuls on the tensor engine
DVE_TAP = 3                         # FMA on DVE (chained after the division)


def _pchunk(i):
    n0 = 128 * i
    return n0, min(128, N - n0)


def _grp(n, size):
    return [list(range(s, min(s + size, n))) for s in range(0, n, size)]


def build_bass(reps=1):
    nc = bacc.Bacc()
    d = {}
    d["xbf"] = nc.declare_dram_parameter("xbf", [BPC, C, N], BD, isOutput=False)
    d["wqkvT"] = nc.declare_dram_parameter("wqkvT", [C, 768], BD, isOutput=False)
    d["wprojT"] = nc.declare_dram_parameter("wprojT", [C, C], BD, isOutput=False)
    d["projb"] = nc.declare_dram_parameter("projb", [C, 1], FD, isOutput=False)
    d["dwcdiag"] = nc.declare_dram_parameter("dwcdiag",
                                             [len(PE_TAPS), CT, 128, 128], BD,
                                             isOutput=False)
    d["dwcw"] = nc.declare_dram_parameter("dwcw", [C, 9], FD, isOutput=False)
    d["dwcb"] = nc.declare_dram_parameter("dwcb", [C, 1], FD, isOutput=False)
    d["out32"] = nc.declare_dram_parameter("out32", [BPC, C, N], FD,
                                           isOutput=True)
    # internal DRAM scratch: bounce for the s2 row-broadcast (SBUF-source
    # DMAs cannot have partition step 0; DRAM-source DMAs can)
    d["s2d"] = nc.dram_tensor("s2d", [NH, N], FD)
    with tile.TileContext(nc) as tc:
        _emit(nc, tc, d, reps)
    nc.finalize()
    return nc


def _emit(nc, tc, d, reps=1):
    import contextlib
    ctx = contextlib.ExitStack()
    with ctx:
        persist = ctx.enter_context(tc.tile_pool(name="persist", bufs=1))
        small = ctx.enter_context(tc.tile_pool(name="small", bufs=2))
        upch = ctx.enter_context(tc.tile_pool(name="upch", bufs=3))
        xbfp = ctx.enter_context(tc.tile_pool(name="xbfp", bufs=2))
        s2fp = ctx.enter_context(tc.tile_pool(name="s2fp", bufs=2))
        ostp = ctx.enter_context(tc.tile_pool(name="ostp", bufs=2))
        # PSUM: "big" 2-bank tiles x2 + "pk" 1-bank tiles x4 = 8 banks
        psum = ctx.enter_context(tc.tile_pool(name="psum", bufs=1,
                                              space="PSUM"))

        t = {}
        t["wqkv"] = persist.tile([128, 2, 768], BD, name="wqkv")
        t["wproj"] = persist.tile([128, 2, C], BD, name="wproj")
        t["projb"] = persist.tile([128, 2, 1], FD, name="projb")
        t["dwcb"] = persist.tile([128, 2, 1], FD, name="dwcb")
        t["dwcw"] = persist.tile([128, 2, 9], FD, name="dwcw")
        t["dwcdiag"] = persist.tile([128, len(PE_TAPS), CT, 128], BD,
                                    name="dwcdiag")
        t["qk"] = persist.tile([128, 4, N], BD, name="qk")
        t["vpad"] = persist.tile([128, CT, PAD * PAD], BD, name="vpad")
        t["v_nm"] = persist.tile([128, NP, NH, 33], BD, name="v_nm")
        t["es1"] = persist.tile([128, NP, NH, 64], BD, name="es1")
        t["w2T"] = persist.tile([128, 4, N], BD, name="w2T")
        t["aT"] = persist.tile([128, CT, A], FD, name="aT")
        t["agBD1"] = persist.tile([128, CT, 4 * A], BD, name="agBD1")
        t["agBD2"] = persist.tile([128, CT, 113], BD, name="agBD2")
        t["onesBD"] = persist.tile([128, 2], BD, name="onesBD")
        t["av_ext"] = persist.tile([128, 4, 64], BD, name="av_ext")
        t["rtile"] = persist.tile([128, 8], FD, name="rtile")
        t["rbraw"] = persist.tile([128, N], FD, name="rbraw")
        t["rbc"] = persist.tile([128, CT, N], BD, name="rbc")
        t["sum"] = persist.tile([128, CT, N], BD, name="sum")

        # --- one-time init: only the zero/ones regions later matmuls read
        # but no per-item pass rewrites (they persist across batch items)
        nc.vector.memset(t["es1"][:, :, :, A:64], 0.0)       # head pads
        nc.vector.memset(t["v_nm"][:, :, :, 32:33], 1.0)     # ones cols
        nc.vector.memset(t["vpad"][:, :, 0:PAD], 0.0)        # top border
        nc.vector.memset(t["vpad"][:, :, 57 * PAD:58 * PAD], 0.0)
        nc.vector.memset(
            t["vpad"][:, :, :].rearrange("p g (r c) -> p g r c", c=PAD)
            [:, :, 1:57, 0:1], 0.0)                          # left border
        nc.vector.memset(
            t["vpad"][:, :, :].rearrange("p g (r c) -> p g r c", c=PAD)
            [:, :, 1:57, 57:58], 0.0)                        # right border
        nc.vector.memset(t["agBD1"][:, :, :], 0.0)
        nc.vector.memset(t["agBD2"][:, :, :], 0.0)
        nc.vector.memset(t["av_ext"][:, :, :], 0.0)
        nc.vector.memset(t["onesBD"][:, :], 0.0)
        nc.vector.memset(t["onesBD"][0:A, 0:1], 1.0)
        nc.vector.memset(t["onesBD"][64:113, 1:2], 1.0)
        # chunk-24 garbage rows of v_nm/es1 are never read (K=64 there)

        nc.sync.dma_start(out=t["wqkv"][:, :, :],
                          in_=d["wqkvT"].rearrange("(t p) f -> p t f", p=128))
        nc.sync.dma_start(out=t["wproj"][:, :, :],
                          in_=d["wprojT"].rearrange("(t p) f -> p t f", p=128))
        nc.sync.dma_start(out=t["projb"][:, :, :],
                          in_=d["projb"].rearrange("(t p) o -> p t o", p=128))
        nc.sync.dma_start(out=t["dwcb"][:, :, :],
                          in_=d["dwcb"].rearrange("(t p) o -> p t o", p=128))
        nc.sync.dma_start(out=t["dwcw"][:, :, :],
                          in_=d["dwcw"].rearrange("(t p) o -> p t o", p=128))
        nc.sync.dma_start(out=t["dwcdiag"][:, :, :, :],
                          in_=d["dwcdiag"].rearrange("t g p f -> p t g f"))

        pools = dict(psum=psum, small=small, upch=upch, xbfp=xbfp,
                     s2fp=s2fp, ostp=ostp)
        for _ in range(reps):
            for b in range(BPC):
                _emit_item(nc, tc, b, d, t, pools)


def _big(pools, nm):
    return pools["psum"].tile([128, 1024], FD, name=nm, tag="pbig", bufs=2)


def _bank(pools, nm):
    return pools["psum"].tile([128, 512], FD, name=nm, tag="pk", bufs=4)


def _emit_item(nc, tc, b, d, t, pools):
    # ---------------- load x ----------------
    xbf = pools["xbfp"].tile([128, CT, N], BD, name="xbf", tag="xbf", bufs=2)
    for g in range(CT):
        nc.sync.dma_start(out=xbf[:, g, :],
                          in_=d["xbf"][b, 128 * g:128 * (g + 1), :])

    # ---------------- qkv projection ----------------
    # qkvT[f, n] = sum_c wqkvT[c, f] x[c, n]; mt 0..3 -> qk, 4..5 -> vpad
    for mt in range(6):
        for js in _grp(NF, 2):
            pt = _big(pools, "pq")
            j0, nj = js[0], len(js)
            for j in js:
                for kt in range(CT):
                    nc.tensor.matmul(
                        pt[:, 512 * (j - j0):512 * (j - j0) + FCH],
                        t["wqkv"][:, kt, 128 * mt:128 * (mt + 1)],
                        xbf[:, kt, FCH * j:FCH * (j + 1)],
                        start=(kt == 0), stop=(kt == CT - 1))
            src = (pt[:, 0:512 * nj].rearrange("p (j f) -> p j f", f=512)
                   [:, :, 0:FCH])
            if mt < 4:
                dst = (t["qk"][:, mt, :].rearrange("p (j f) -> p j f", f=FCH)
                       [:, j0:j0 + nj, :])
                if (j0 // 2) % 2 == 0:
                    nc.vector.tensor_copy(out=dst, in_=src)
                else:
                    nc.scalar.copy(out=dst, in_=src)
            else:
                g = mt - 4
                rows = (t["vpad"][:, g, :]
                        .rearrange("p (r c) -> p r c", c=PAD)
                        [:, 1 + 8 * j0:1 + 8 * j0 + 8 * nj, 1:57]
                        .rearrange("p (j r) c -> p j r c", r=8))
                nc.vector.tensor_copy(
                    out=rows, in_=src.rearrange("p j (r c) -> p j r c", c=56))

    # ---------------- pooling -> agent blocks ----------------
    for g in range(CT):
        a1 = pools["small"].tile([128, 56 * 7], FD, name="a1", tag="a1")
        nc.vector.tensor_reduce(
            out=a1[:, :],
            in_=xbf[:, g, :].rearrange("p (h wb wi) -> p h wb wi", wb=7,
                                       wi=8),
            axis=mybir.AxisListType.X, op=OP.add)
        nc.vector.tensor_reduce(
            out=t["aT"][:, g, :],
            in_=a1[:, :].rearrange("p (hb hi wb) -> p hb wb hi", hb=7, hi=8,
                                   wb=7),
            axis=mybir.AxisListType.X, op=OP.add)
        # the two 8-sums fold into one /64; scale applied during the copies
        for hp in range(4):
            nc.vector.tensor_scalar_mul(
                out=t["agBD1"][32 * hp:32 * (hp + 1), g, A * hp:A * (hp + 1)],
                in0=t["aT"][32 * hp:32 * (hp + 1), g, :], scalar1=1.0 / 64.0)
        for q in range(2):
            nc.vector.tensor_scalar_mul(
                out=t["agBD2"][64 * q:64 * q + 32, g, 0:A],
                in0=t["aT"][64 * q:64 * q + 32, g, :], scalar1=1.0 / 64.0)
            nc.vector.tensor_scalar_mul(
                out=t["agBD2"][64 * q + 32:64 * q + 64, g, 64:113],
                in0=t["aT"][64 * q + 32:64 * q + 64, g, :], scalar1=1.0 / 64.0)

    # ---- stage-1 scores/exp + v_nm + agent_v, interleaved per chunk-group
    # (keeps PE on v_nm/agv matmuls while ACT runs the stage-1 exps, and DVE
    # on v_nm copies; the agent_v accumulation trails one group behind)
    agv = _bank(pools, "agv")
    agv3 = agv[:, 0:264].rearrange("p (q f) -> p q f", f=66)

    def s1_group(g, iis):
        pt = _big(pools, "p1")
        i0 = iis[0]
        for i in iis:
            n0, sz = _pchunk(i)
            nc.tensor.matmul(
                pt[0:sz, 512 * (i - i0):512 * (i - i0) + 196],
                t["qk"][:, 2 + g, n0:n0 + sz],
                t["agBD1"][:, g, :], start=True, stop=True)
        full = [i for i in iis if _pchunk(i)[1] == 128]
        if full:
            nfull = len(full)
            src = (pt[:, 512 * (full[0] - i0):
                      512 * (full[0] - i0) + 512 * nfull]
                   .rearrange("p (j f) -> p j f", f=512)[:, :, 0:196]
                   .rearrange("p j (h a) -> p j h a", a=A))
            nc.scalar.activation(
                out=t["es1"][:, full[0]:full[0] + nfull, 4 * g:4 * g + 4, 0:A],
                in_=src, func=FX.Exp, scale=SCALE)
        for i in iis:
            n0, sz = _pchunk(i)
            if sz == 128:
                continue
            src = (pt[0:sz, 512 * (i - i0):512 * (i - i0) + 196]
                   .rearrange("p (h a) -> p h a", a=A))
            nc.scalar.activation(
                out=t["es1"][0:sz, i, 4 * g:4 * g + 4, 0:A],
                in_=src, func=FX.Exp, scale=SCALE)

    def v_group(iis):
        pt = _big(pools, "pv")
        i0, nj = iis[0], len(iis)
        for i in iis:
            n0, sz = _pchunk(i)
            for kt in range(CT):
                nc.tensor.matmul(
                    pt[0:sz, 512 * (i - i0):512 * (i - i0) + C],
                    xbf[:, kt, n0:n0 + sz],
                    t["wqkv"][:, kt, 512:768],
                    start=(kt == 0), stop=(kt == CT - 1))
        src = (pt[:, 0:512 * nj].rearrange("p (j f) -> p j f", f=512)
               [:, :, 0:C].rearrange("p j (h dd) -> p j h dd", dd=32))
        nc.vector.tensor_copy(out=t["v_nm"][:, i0:i0 + nj, :, 0:32], in_=src)

    def agv_group(iis):
        for i in iis:
            n0, sz = _pchunk(i)
            for p in range(4):
                nc.tensor.matmul(
                    agv3[:, p, :],
                    t["es1"][0:sz, i, 2 * p:2 * p + 2, :].rearrange(
                        "p h dd -> p (h dd)"),
                    t["v_nm"][0:sz, i, 2 * p:2 * p + 2, :].rearrange(
                        "p h dd -> p (h dd)"),
                    start=(i == 0), stop=(i == NP - 1))

    groups = _grp(NP, 2)
    for k, iis in enumerate(groups):
        s1_group(0, iis)
        s1_group(1, iis)
        v_group(iis)
        if k > 0:
            agv_group(groups[k - 1])
    agv_group(groups[-1])

    nc.vector.reciprocal(out=t["rtile"][0:A, 0:4], in_=agv3[0:A, :, 32])
    nc.vector.reciprocal(out=t["rtile"][64:113, 4:8], in_=agv3[64:113, :, 65])
    with nc.allow_low_precision("bf16 attention weights"):
        nc.vector.tensor_tensor(
            out=t["av_ext"][0:A, :, 0:32], in0=agv3[0:A, :, 0:32],
            in1=t["rtile"][0:A, 0:4].unsqueeze(2).to_broadcast([A, 4, 32]),
            op=OP.mult)
        nc.vector.tensor_tensor(
            out=t["av_ext"][64:113, :, 32:64], in0=agv3[64:113, :, 33:65],
            in1=t["rtile"][64:113, 4:8].unsqueeze(2).to_broadcast([A, 4, 32]),
            op=OP.mult)

    # ------- stage 2: score/exp + denominator chains for both groups -----
    for a in range(2):
        # scores + exp for pairs 2a (rows 0:64) and 2a+1 (rows 64:128);
        # interleaved so the PE overlaps them on disjoint row groups
        for js in _grp(NF, 2):
            j0, nj = js[0], len(js)
            pts = [_big(pools, "p2"), _big(pools, "p2")]
            for j in js:
                for q in range(2):
                    p = 2 * a + q
                    nc.tensor.matmul(
                        pts[q][0:113, 512 * (j - j0):512 * (j - j0) + FCH],
                        t["agBD2"][64 * q:64 * (q + 1), a, :],
                        t["qk"][64 * q:64 * (q + 1), a,
                                FCH * j:FCH * (j + 1)],
                        start=True, stop=True)
            for q in range(2):
                p = 2 * a + q
                src = (pts[q][0:113, 0:512 * nj]
                       .rearrange("p (j f) -> p j f", f=512)[:, :, 0:FCH])
                dst = (t["w2T"][0:113, p, :]
                       .rearrange("p (j f) -> p j f", f=FCH)
                       [:, j0:j0 + nj, :])
                nc.scalar.activation(out=dst, in_=src, func=FX.Exp,
                                     scale=SCALE)
        # denominators for this pair group: ones-matmul -> PSUM rows
        # {0,1,64,65}; 1/s2 computed as exp(-ln(s2)) entirely on ACT (Ln
        # replaces the PSUM->SBUF staging copy, Exp runs once per group on
        # the 128-row broadcast; both funcs live in one activation table).
        for js in _grp(NF, 2):
            j0, nj = js[0], len(js)
            pt = _big(pools, "po")
            for j in js:
                for q in range(2):
                    nc.tensor.matmul(
                        pt[64 * q:64 * q + 2,
                           512 * (j - j0):512 * (j - j0) + FCH],
                        t["onesBD"][0:113, :],
                        t["w2T"][0:113, 2 * a + q, FCH * j:FCH * (j + 1)],
                        start=True, stop=True)
            s2f = pools["s2fp"].tile([128, 2, FCH], FD, name="s2f",
                                     tag="s2f", bufs=2)
            pt_ap = pt[:, :]
            s2f_ap = s2f[:, :, :]
            for q in range(2):
                src = bass.AP(tensor=pt_ap.tensor,
                              offset=pt_ap.offset + (64 * q) * 1024,
                              ap=[[1024, 2], [512, nj], [1, FCH]])
                dst = bass.AP(tensor=s2f_ap.tensor,
                              offset=s2f_ap.offset + (64 * q) * (2 * FCH),
                              ap=[[2 * FCH, 2], [FCH, nj], [1, FCH]])
                nc.scalar.activation(out=dst, in_=src, func=FX.Ln)
                # rows {64q, 64q+1} hold heads 2*(2a+q), 2*(2a+q)+1
                ddst = bass.AP(tensor=d["s2d"],
                               offset=(4 * a + 2 * q) * N + FCH * j0,
                               ap=[[N, 2], [FCH, nj], [1, FCH]])
                dsrc = bass.AP(tensor=s2f_ap.tensor,
                               offset=s2f_ap.offset + (64 * q) * (2 * FCH),
                               ap=[[2 * FCH, 2], [FCH, nj], [1, FCH]])
                nc.scalar.dma_start(out=ddst, in_=dsrc)
        # broadcast ln(s2) to the 32 channel rows of each head (DRAM bounce;
        # heads of group a are exactly channel-tile g = a)
        for h in range(4 * a, 4 * a + 4):
            src = bass.AP(tensor=d["s2d"], offset=h * N, ap=[[0, 32], [1, N]])
            nc.scalar.dma_start(out=t["rbraw"][32 * (h % 4):32 * (h % 4) + 32,
                                               :], in_=src)
        with nc.allow_low_precision("bf16 softmax reciprocal"):
            nc.scalar.activation(out=t["rbc"][:, a, :], in_=t["rbraw"][:, :],
                                 func=FX.Exp, scale=-1.0)


    # ---------------- dwc + apply + combine ----------------
    for g in range(2):
        vp = t["vpad"][:, g, :].rearrange("p (r c) -> p r c", c=PAD)
        for j in range(NF):
            pd = _bank(pools, "pd")
            pa = _bank(pools, "pa")
            # PE taps (diag matmuls accumulate) + apply (row-split pairs)
            for k, ti in enumerate(PE_TAPS):
                dy, dx = TAPS[ti]
                nc.tensor.matmul(
                    pd[:, 0:FCH].rearrange("p (r c) -> p r c", c=56),
                    t["dwcdiag"][:, k, g, :],
                    vp[:, 8 * j + dy:8 * j + dy + 8, dx:dx + 56],
                    start=(k == 0), stop=(k == len(PE_TAPS) - 1))
            for q in range(2):
                nc.tensor.matmul(
                    pa[64 * q:64 * (q + 1), 0:FCH],
                    t["av_ext"][0:113, 2 * g + q, :],
                    t["w2T"][0:113, 2 * g + q, FCH * j:FCH * (j + 1)],
                    start=True, stop=True)
            upt = pools["upch"].tile([128, FCH], BD, name="up", tag="up")
            with nc.allow_low_precision("bf16 activations"):
                # u' = u * (1/s2)          (DVE, PSUM read)
                nc.vector.tensor_tensor(
                    out=upt[:, :], in0=pa[:, 0:FCH],
                    in1=t["rbc"][:, g, FCH * j:FCH * (j + 1)], op=OP.mult)
                # u' += dve-tap            (DVE, SBUF FMA)
                dy, dx = TAPS[DVE_TAP]
                nc.vector.scalar_tensor_tensor(
                    out=upt[:, :].rearrange("p (r c) -> p r c", c=56),
                    in0=vp[:, 8 * j + dy:8 * j + dy + 8, dx:dx + 56],
                    scalar=t["dwcw"][:, g, DVE_TAP:DVE_TAP + 1],
                    in1=upt[:, :].rearrange("p (r c) -> p r c", c=56),
                    op0=OP.mult, op1=OP.add)
                # sum = (dwc_pe + dwc_b) + u'   (DVE, PSUM read)
                nc.vector.scalar_tensor_tensor(
                    out=t["sum"][:, g, FCH * j:FCH * (j + 1)],
                    in0=pd[:, 0:FCH], scalar=t["dwcb"][:, g, :],
                    in1=upt[:, :], op0=OP.add, op1=OP.add)

    # ---------------- proj + bias + residual ----------------
    for mt in range(CT):
        for js in _grp(NF, 2):
            j0, nj = js[0], len(js)
            pp = _big(pools, "pp")
            for j in js:
                for kt in range(CT):
                    nc.tensor.matmul(
                        pp[:, 512 * (j - j0):512 * (j - j0) + FCH],
                        t["wproj"][:, kt, 128 * mt:128 * (mt + 1)],
                        t["sum"][:, kt, FCH * j:FCH * (j + 1)],
                        start=(kt == 0), stop=(kt == CT - 1))
            src = (pp[:, 0:512 * nj].rearrange("p (j f) -> p j f", f=512)
                   [:, :, 0:FCH])
            ost = pools["ostp"].tile([128, 2, FCH], FD, name="ost",
                                     tag="ost", bufs=2)
            resid = (xbf[:, mt, FCH * j0:FCH * (j0 + nj)]
                     .rearrange("p (j f) -> p j f", f=FCH))
            nc.vector.scalar_tensor_tensor(
                out=ost[:, 0:nj, :], in0=src, scalar=t["projb"][:, mt, :],
                in1=resid, op0=OP.add, op1=OP.add)
            nc.gpsimd.dma_start(
                out=d["out32"][b, 128 * mt:128 * (mt + 1),
                               FCH * j0:FCH * (j0 + nj)],
                in_=ost[:, 0:nj, :])


def host_prep(x, qkv_w, proj_w, proj_b, dwc_w, dwc_b):
    xf = np.ascontiguousarray(x.reshape(B, C, N), dtype=FP32)
    wqkvT = np.ascontiguousarray(np.asarray(qkv_w, FP32).T.astype(BF16))
    wprojT = np.ascontiguousarray(np.asarray(proj_w, FP32).T.astype(BF16))
    projb = np.ascontiguousarray(np.asarray(proj_b, FP32).reshape(C, 1))
    dwcb = np.ascontiguousarray(np.asarray(dwc_b, FP32).reshape(C, 1))
    w33 = np.asarray(dwc_w, FP32).reshape(C, 9)
    dd = np.zeros((len(PE_TAPS), CT, 128, 128), FP32)
    for k, ti in enumerate(PE_TAPS):
        for g in range(CT):
            np.fill_diagonal(dd[k, g], w33[128 * g:128 * (g + 1), ti])
    dwcdiag = np.ascontiguousarray(dd.astype(BF16))
    dwcw = np.ascontiguousarray(w33)
    maps = []
    for c in range(NCORES):
        xs = xf[BPC * c:BPC * (c + 1)]
        maps.append(dict(
            xbf=np.ascontiguousarray(xs.astype(BF16)),
            wqkvT=wqkvT, wprojT=wprojT, projb=projb,
            dwcdiag=dwcdiag, dwcw=dwcw, dwcb=dwcb,
        ))
    return maps


_NC_CACHE = {}


def kernel(x, qkv_w, proj_w, proj_b, dwc_w, dwc_b, trace=False):
    if "nc" not in _NC_CACHE:
        _NC_CACHE["nc"] = build_bass()
    nc = _NC_CACHE["nc"]
    maps = host_prep(np.asarray(x), np.asarray(qkv_w), np.asarray(proj_w),
                     np.asarray(proj_b), np.asarray(dwc_w), np.asarray(dwc_b))
    res = run_bass_kernel_spmd(nc, maps, core_ids=list(range(NCORES)),
                               trace=trace)
    outs = [r["out32"].reshape(BPC, C, HH, WW) for r in res.results]
    full = np.concatenate(outs, axis=0).astype(np.float32)
    if trace:
        return full, res
    return full



# revision 31
# speedup vs baseline: 1.7308x; 1.0622x over previous
"""AgentAttention Trainium2 kernel.

Sharding: data-parallel over batch B=16 across 8 NeuronCores (2 items/core),
no collectives. Per batch item (C=256, N=56*56=3136, 8 heads, hd=32, 49
agents), everything in bf16 matmuls with fp32 PSUM accumulation:

  qkvT[768,N]  = qkv_w @ x                       (PE, feature-major)
  a[C,49]      = 8x8 avg-pool of x               (DVE reduces)
  S1T[N,(h,49)]= kT-chunk.T @ agentT-blockdiag   (PE, 4 heads per pass)
  expS1T       = exp(scale*S1T)                  (ACT; scores are O(0.3) so
                                                  softmax needs no max-subtract)
  agent_v      = expS1T.T @ [v_nm | 1]           (PE; ones-col = denominator)
  w2T[49,N]/h  = exp(scale * agBD2.T @ qT)       (PE+ACT, 2 heads per pass)
  s2           = onesT @ w2T                     (PE; stage-2 denominators)
  u[C,N]       = av_blockdiag.T @ w2T            (PE)
  x_att        = u * bcast(1/s2)                 (DRAM-bounce row-bcast + DVE)
  dwc          = 3x3 depthwise conv: 7 taps as accumulating diag matmuls on
                 PE, 1 tap each on DVE/GPSIMD as per-partition-scalar FMA
  out          = proj_w @ (x_att + dwc + b) + proj_b + x    (PE + DVE)

The stage-2 tail (denominators -> DRAM-bounce broadcast -> reciprocal ->
combine) is pipelined per pair-group: group a's chain runs while group
(a+1)'s scores/exp are still on PE/ACT, and combine for channel-tile g==a
starts as soon as its rbc slice is ready.
"""

import sys

for _p in ("/opt/trn_rl_repo", "/opt/trn_rl_repo/concourse"):
    if _p not in sys.path:
        sys.path.insert(0, _p)

import numpy as np
import ml_dtypes

import concourse.bass as bass
import concourse.bacc as bacc
import concourse.mybir as mybir
import concourse.tile as tile
from concourse.bass_utils import run_bass_kernel_spmd

BF16 = ml_dtypes.bfloat16
FP32 = np.float32

B, C, HH, WW = 16, 256, 56, 56
N = HH * WW            # 3136
NH, HD, A = 8, 32, 49
SCALE = float(HD) ** -0.5
NCORES = 8
BPC = B // NCORES      # 2
CT = 2                 # 128-channel tiles
NP = 25                # ceil(N/128); last chunk is 64
FCH = 448              # free-dim chunk = 8 image rows
NF = 7
PAD = 58
BD = mybir.dt.bfloat16
FD = mybir.dt.float32
HD16 = mybir.dt.float16
FX = mybir.ActivationFunctionType
OP = mybir.AluOpType

TAPS = [(dy, dx) for dy in range(3) for dx in range(3)]
PE_TAPS = [0, 1, 2, 4, 5, 6, 7, 8]  # diag matmuls on the tensor engine
DVE_TAP = 3                         # FMA on DVE (chained after the division)


def _pchunk(i):
    n0 = 128 * i
    return n0, min(128, N - n0)


def _grp(n, size):
    return [list(range(s, min(s + size, n))) for s in range(0, n, size)]


def build_bass(reps=1):
    nc = bacc.Bacc()
    d = {}
    d["xbf"] = nc.declare_dram_parameter("xbf", [BPC, C, N], BD, isOutput=False)
    d["wqkvT"] = nc.declare_dram_parameter("wqkvT", [C, 768], BD, isOutput=False)
    d["wprojT"] = nc.declare_dram_parameter("wprojT", [C, C], BD, isOutput=False)
    d["projb"] = nc.declare_dram_parameter("projb", [C, 1], FD, isOutput=False)
    d["dwcdiag"] = nc.declare_dram_parameter("dwcdiag",
                                             [len(PE_TAPS), CT, 128, 128], BD,
                                             isOutput=False)
    d["dwcw"] = nc.declare_dram_parameter("dwcw", [C, 9], FD, isOutput=False)
    d["dwcb"] = nc.declare_dram_parameter("dwcb", [C, 1], FD, isOutput=False)
    d["out32"] = nc.declare_dram_parameter("out32", [BPC, C, N], FD,
                                           isOutput=True)
    # internal DRAM scratch: bounce for the s2 row-broadcast (SBUF-source
    # DMAs cannot have partition step 0; DRAM-source DMAs can)
    d["s2d"] = nc.dram_tensor("s2d", [NH, N], HD16)
    with tile.TileContext(nc) as tc:
        _emit(nc, tc, d, reps)
    nc.finalize()
    return nc


def _emit(nc, tc, d, reps=1):
    import contextlib
    ctx = contextlib.ExitStack()
    with ctx:
        persist = ctx.enter_context(tc.tile_pool(name="persist", bufs=1))
        small = ctx.enter_context(tc.tile_pool(name="small", bufs=2))
        upch = ctx.enter_context(tc.tile_pool(name="upch", bufs=3))
        xbfp = ctx.enter_context(tc.tile_pool(name="xbfp", bufs=2))
        s2fp = ctx.enter_context(tc.tile_pool(name="s2fp", bufs=2))
        ostp = ctx.enter_context(tc.tile_pool(name="ostp", bufs=2))
        # PSUM: "big" 2-bank tiles x2 + "pk" 1-bank tiles x4 = 8 banks
        psum = ctx.enter_context(tc.tile_pool(name="psum", bufs=1,
                                              space="PSUM"))

        t = {}
        t["wqkv"] = persist.tile([128, 2, 768], BD, name="wqkv")
        t["wproj"] = persist.tile([128, 2, C], BD, name="wproj")
        t["projb"] = persist.tile([128, 2, 1], FD, name="projb")
        t["dwcb"] = persist.tile([128, 2, 1], FD, name="dwcb")
        t["dwcw"] = persist.tile([128, 2, 9], FD, name="dwcw")
        t["dwcdiag"] = persist.tile([128, len(PE_TAPS), CT, 128], BD,
                                    name="dwcdiag")
        t["qk"] = persist.tile([128, 4, N], BD, name="qk")
        t["vpad"] = persist.tile([128, CT, PAD * PAD], BD, name="vpad")
        t["v_nm"] = persist.tile([128, NP, NH, 33], BD, name="v_nm")
        t["es1"] = persist.tile([128, NP, NH, 64], BD, name="es1")
        t["w2T"] = persist.tile([128, 4, N], BD, name="w2T")
        t["aT"] = persist.tile([128, CT, A], FD, name="aT")
        t["agBD1"] = persist.tile([128, CT, 4 * A], BD, name="agBD1")
        t["agBD2"] = persist.tile([128, CT, 113], BD, name="agBD2")
        t["onesBD"] = persist.tile([128, 2], BD, name="onesBD")
        t["av_ext"] = persist.tile([128, 4, 64], BD, name="av_ext")
        t["rtile"] = persist.tile([128, 8], FD, name="rtile")
        t["rbraw"] = persist.tile([128, N], HD16, name="rbraw")
        t["rbc"] = persist.tile([128, CT, N], BD, name="rbc")
        t["sum"] = persist.tile([128, CT, N], BD, name="sum")

        # --- one-time init: only the zero/ones regions later matmuls read
        # but no per-item pass rewrites (they persist across batch items)
        nc.vector.memset(t["es1"][:, :, :, A:64], 0.0)       # head pads
        nc.vector.memset(t["v_nm"][:, :, :, 32:33], 1.0)     # ones cols
        nc.vector.memset(t["vpad"][:, :, 0:PAD], 0.0)        # top border
        nc.vector.memset(t["vpad"][:, :, 57 * PAD:58 * PAD], 0.0)
        nc.vector.memset(
            t["vpad"][:, :, :].rearrange("p g (r c) -> p g r c", c=PAD)
            [:, :, 1:57, 0:1], 0.0)                          # left border
        nc.vector.memset(
            t["vpad"][:, :, :].rearrange("p g (r c) -> p g r c", c=PAD)
            [:, :, 1:57, 57:58], 0.0)                        # right border
        nc.vector.memset(t["agBD1"][:, :, :], 0.0)
        nc.vector.memset(t["agBD2"][:, :, :], 0.0)
        nc.vector.memset(t["av_ext"][:, :, :], 0.0)
        nc.vector.memset(t["onesBD"][:, :], 0.0)
        nc.vector.memset(t["onesBD"][0:A, 0:1], 1.0)
        nc.vector.memset(t["onesBD"][64:113, 1:2], 1.0)
        # chunk-24 garbage rows of v_nm/es1 are never read (K=64 there)

        nc.sync.dma_start(out=t["wqkv"][:, :, :],
                          in_=d["wqkvT"].rearrange("(t p) f -> p t f", p=128))
        nc.sync.dma_start(out=t["wproj"][:, :, :],
                          in_=d["wprojT"].rearrange("(t p) f -> p t f", p=128))
        nc.sync.dma_start(out=t["projb"][:, :, :],
                          in_=d["projb"].rearrange("(t p) o -> p t o", p=128))
        nc.sync.dma_start(out=t["dwcb"][:, :, :],
                          in_=d["dwcb"].rearrange("(t p) o -> p t o", p=128))
        nc.sync.dma_start(out=t["dwcw"][:, :, :],
                          in_=d["dwcw"].rearrange("(t p) o -> p t o", p=128))
        nc.sync.dma_start(out=t["dwcdiag"][:, :, :, :],
                          in_=d["dwcdiag"].rearrange("t g p f -> p t g f"))

        pools = dict(psum=psum, small=small, upch=upch, xbfp=xbfp,
                     s2fp=s2fp, ostp=ostp)
        for _ in range(reps):
            for b in range(BPC):
                _emit_item(nc, tc, b, d, t, pools)


def _big(pools, nm):
    return pools["psum"].tile([128, 1024], FD, name=nm, tag="pbig", bufs=2)


def _bank(pools, nm):
    return pools["psum"].tile([128, 512], FD, name=nm, tag="pk", bufs=4)


def _emit_item(nc, tc, b, d, t, pools):
    # ---------------- load x ----------------
    xbf = pools["xbfp"].tile([128, CT, N], BD, name="xbf", tag="xbf", bufs=2)
    for g in range(CT):
        nc.sync.dma_start(out=xbf[:, g, :],
                          in_=d["xbf"][b, 128 * g:128 * (g + 1), :])

    # ---------------- qkv projection ----------------
    # qkvT[f, n] = sum_c wqkvT[c, f] x[c, n]; mt 0..3 -> qk, 4..5 -> vpad
    # (kt outer so the stationary weight tile is reused across j)
    for mt in range(6):
        for js in _grp(NF, 2):
            pt = _big(pools, "pq")
            j0, nj = js[0], len(js)
            for kt in range(CT):
                for j in js:
                    nc.tensor.matmul(
                        pt[:, 512 * (j - j0):512 * (j - j0) + FCH],
                        t["wqkv"][:, kt, 128 * mt:128 * (mt + 1)],
                        xbf[:, kt, FCH * j:FCH * (j + 1)],
                        start=(kt == 0), stop=(kt == CT - 1))
            src = (pt[:, 0:512 * nj].rearrange("p (j f) -> p j f", f=512)
                   [:, :, 0:FCH])
            if mt < 4:
                dst = (t["qk"][:, mt, :].rearrange("p (j f) -> p j f", f=FCH)
                       [:, j0:j0 + nj, :])
                if (j0 // 2) % 2 == 0:
                    nc.vector.tensor_copy(out=dst, in_=src)
                else:
                    nc.scalar.copy(out=dst, in_=src)
            else:
                g = mt - 4
                rows = (t["vpad"][:, g, :]
                        .rearrange("p (r c) -> p r c", c=PAD)
                        [:, 1 + 8 * j0:1 + 8 * j0 + 8 * nj, 1:57]
                        .rearrange("p (j r) c -> p j r c", r=8))
                nc.vector.tensor_copy(
                    out=rows, in_=src.rearrange("p j (r c) -> p j r c", c=56))

    # ---------------- pooling -> agent blocks ----------------
    for g in range(CT):
        a1 = pools["small"].tile([128, 56 * 7], FD, name="a1", tag="a1")
        nc.vector.tensor_reduce(
            out=a1[:, :],
            in_=xbf[:, g, :].rearrange("p (h wb wi) -> p h wb wi", wb=7,
                                       wi=8),
            axis=mybir.AxisListType.X, op=OP.add)
        nc.vector.tensor_reduce(
            out=t["aT"][:, g, :],
            in_=a1[:, :].rearrange("p (hb hi wb) -> p hb wb hi", hb=7, hi=8,
                                   wb=7),
            axis=mybir.AxisListType.X, op=OP.add)
        # the two 8-sums fold into one /64; scale applied during the copies
        for hp in range(4):
            nc.vector.tensor_scalar_mul(
                out=t["agBD1"][32 * hp:32 * (hp + 1), g, A * hp:A * (hp + 1)],
                in0=t["aT"][32 * hp:32 * (hp + 1), g, :], scalar1=1.0 / 64.0)
        for q in range(2):
            nc.vector.tensor_scalar_mul(
                out=t["agBD2"][64 * q:64 * q + 32, g, 0:A],
                in0=t["aT"][64 * q:64 * q + 32, g, :], scalar1=1.0 / 64.0)
            nc.vector.tensor_scalar_mul(
                out=t["agBD2"][64 * q + 32:64 * q + 64, g, 64:113],
                in0=t["aT"][64 * q + 32:64 * q + 64, g, :], scalar1=1.0 / 64.0)

    # ---- stage-1 scores/exp + v_nm + agent_v, interleaved per chunk-group
    # (keeps PE on v_nm/agv matmuls while ACT runs the stage-1 exps, and DVE
    # on v_nm copies; the agent_v accumulation trails one group behind)
    agv = _bank(pools, "agv")
    agv3 = agv[:, 0:264].rearrange("p (q f) -> p q f", f=66)

    def s1_group(g, iis):
        pt = _big(pools, "p1")
        i0 = iis[0]
        for i in iis:
            n0, sz = _pchunk(i)
            nc.tensor.matmul(
                pt[0:sz, 512 * (i - i0):512 * (i - i0) + 196],
                t["qk"][:, 2 + g, n0:n0 + sz],
                t["agBD1"][:, g, :], start=True, stop=True)
        full = [i for i in iis if _pchunk(i)[1] == 128]
        if full:
            nfull = len(full)
            src = (pt[:, 512 * (full[0] - i0):
                      512 * (full[0] - i0) + 512 * nfull]
                   .rearrange("p (j f) -> p j f", f=512)[:, :, 0:196]
                   .rearrange("p j (h a) -> p j h a", a=A))
            nc.scalar.activation(
                out=t["es1"][:, full[0]:full[0] + nfull, 4 * g:4 * g + 4, 0:A],
                in_=src, func=FX.Exp, scale=SCALE)
        for i in iis:
            n0, sz = _pchunk(i)
            if sz == 128:
                continue
            src = (pt[0:sz, 512 * (i - i0):512 * (i - i0) + 196]
                   .rearrange("p (h a) -> p h a", a=A))
            nc.scalar.activation(
                out=t["es1"][0:sz, i, 4 * g:4 * g + 4, 0:A],
                in_=src, func=FX.Exp, scale=SCALE)

    def v_group(iis):
        pt = _big(pools, "pv")
        i0, nj = iis[0], len(iis)
        for i in iis:
            n0, sz = _pchunk(i)
            for kt in range(CT):
                nc.tensor.matmul(
                    pt[0:sz, 512 * (i - i0):512 * (i - i0) + C],
                    xbf[:, kt, n0:n0 + sz],
                    t["wqkv"][:, kt, 512:768],
                    start=(kt == 0), stop=(kt == CT - 1))
        src = (pt[:, 0:512 * nj].rearrange("p (j f) -> p j f", f=512)
               [:, :, 0:C].rearrange("p j (h dd) -> p j h dd", dd=32))
        nc.vector.tensor_copy(out=t["v_nm"][:, i0:i0 + nj, :, 0:32], in_=src)

    def agv_group(iis):
        for i in iis:
            n0, sz = _pchunk(i)
            for p in range(4):
                nc.tensor.matmul(
                    agv3[:, p, :],
                    t["es1"][0:sz, i, 2 * p:2 * p + 2, :].rearrange(
                        "p h dd -> p (h dd)"),
                    t["v_nm"][0:sz, i, 2 * p:2 * p + 2, :].rearrange(
                        "p h dd -> p (h dd)"),
                    start=(i == 0), stop=(i == NP - 1))

    groups = _grp(NP, 2)
    for k, iis in enumerate(groups):
        s1_group(0, iis)
        s1_group(1, iis)
        v_group(iis)
        if k > 0:
            agv_group(groups[k - 1])
    agv_group(groups[-1])

    nc.vector.reciprocal(out=t["rtile"][0:A, 0:4], in_=agv3[0:A, :, 32])
    nc.vector.reciprocal(out=t["rtile"][64:113, 4:8], in_=agv3[64:113, :, 65])
    with nc.allow_low_precision("bf16 attention weights"):
        nc.vector.tensor_tensor(
            out=t["av_ext"][0:A, :, 0:32], in0=agv3[0:A, :, 0:32],
            in1=t["rtile"][0:A, 0:4].unsqueeze(2).to_broadcast([A, 4, 32]),
            op=OP.mult)
        nc.vector.tensor_tensor(
            out=t["av_ext"][64:113, :, 32:64], in0=agv3[64:113, :, 33:65],
            in1=t["rtile"][64:113, 4:8].unsqueeze(2).to_broadcast([A, 4, 32]),
            op=OP.mult)

    # ------- stage 2: score/exp + denominator chains for both groups -----
    for a in range(2):
        # scores + exp for pairs 2a (rows 0:64) and 2a+1 (rows 64:128);
        # interleaved so the PE overlaps them on disjoint row groups
        for js in _grp(NF, 2):
            j0, nj = js[0], len(js)
            pts = [_big(pools, "p2"), _big(pools, "p2")]
            for q in range(2):
                for j in js:
                    nc.tensor.matmul(
                        pts[q][0:113, 512 * (j - j0):512 * (j - j0) + FCH],
                        t["agBD2"][64 * q:64 * (q + 1), a, :],
                        t["qk"][64 * q:64 * (q + 1), a,
                                FCH * j:FCH * (j + 1)],
                        start=True, stop=True)
            for q in range(2):
                p = 2 * a + q
                src = (pts[q][0:113, 0:512 * nj]
                       .rearrange("p (j f) -> p j f", f=512)[:, :, 0:FCH])
                dst = (t["w2T"][0:113, p, :]
                       .rearrange("p (j f) -> p j f", f=FCH)
                       [:, j0:j0 + nj, :])
                nc.scalar.activation(out=dst, in_=src, func=FX.Exp,
                                     scale=SCALE)
        # denominators for this pair group: one ones-matmul per chunk covers
        # both head pairs (w2T slots 2a, 2a+1 via the rhs slot dim); 1/s2 is
        # exp(-ln(s2)) entirely on ACT (Ln replaces the PSUM->SBUF staging
        # copy, Exp runs once per group on the 128-row broadcast; both funcs
        # live in one activation table). out[e, q, n] = denom of head 4a+2q+e.
        for js in _grp(NF, 2):
            j0, nj = js[0], len(js)
            pt = _big(pools, "po")
            for j in js:
                for q in range(2):
                    nc.tensor.matmul(
                        pt[64 * q:64 * q + 2,
                           512 * (j - j0):512 * (j - j0) + FCH],
                        t["onesBD"][0:113, :],
                        t["w2T"][0:113, 2 * a + q, FCH * j:FCH * (j + 1)],
                        start=True, stop=True)
            s2f = pools["s2fp"].tile([128, 2, FCH], HD16, name="s2f",
                                     tag="s2f", bufs=2)
            pt_ap = pt[:, :]
            s2f_ap = s2f[:, :, :]
            for q in range(2):
                src = bass.AP(tensor=pt_ap.tensor,
                              offset=pt_ap.offset + (64 * q) * 1024,
                              ap=[[1024, 2], [512, nj], [1, FCH]])
                dst = bass.AP(tensor=s2f_ap.tensor,
                              offset=s2f_ap.offset + (64 * q) * (2 * FCH),
                              ap=[[2 * FCH, 2], [FCH, nj], [1, FCH]])
                with nc.allow_low_precision("fp16 log-denominators"):
                    nc.scalar.activation(out=dst, in_=src, func=FX.Ln)
                ddst = bass.AP(tensor=d["s2d"],
                               offset=(4 * a + 2 * q) * N + FCH * j0,
                               ap=[[N, 2], [FCH, nj], [1, FCH]])
                dsrc = bass.AP(tensor=s2f_ap.tensor,
                               offset=s2f_ap.offset + (64 * q) * (2 * FCH),
                               ap=[[2 * FCH, 2], [FCH, nj], [1, FCH]])
                nc.sync.dma_start(out=ddst, in_=dsrc)
        # broadcast ln(s2) to the 32 channel rows of each head (DRAM bounce;
        # heads of group a are exactly channel-tile g = a)
        for h in range(4 * a, 4 * a + 4):
            src = bass.AP(tensor=d["s2d"], offset=h * N, ap=[[0, 32], [1, N]])
            nc.sync.dma_start(out=t["rbraw"][32 * (h % 4):32 * (h % 4) + 32,
                                             :], in_=src)
        with nc.allow_low_precision("bf16 softmax reciprocal"):
            nc.scalar.activation(out=t["rbc"][:, a, :], in_=t["rbraw"][:, :],
                                 func=FX.Exp, scale=-1.0)


    # ---------------- dwc + apply + combine ----------------
    # j-chunks in halves of 4+3 so each diag tap weight is loaded once per
    # half (tap-outer loop); pd pairs pack into "big" 2-bank tiles while the
    # pa (stage-2 apply) matmuls use the single-bank tiles.
    for g in range(2):
        vp = t["vpad"][:, g, :].rearrange("p (r c) -> p r c", c=PAD)
        for half in _grp(NF, 4):
            pds = [_big(pools, "pd") for _ in range((len(half) + 1) // 2)]
            for k, ti in enumerate(PE_TAPS):
                dy, dx = TAPS[ti]
                for idx, j in enumerate(half):
                    nc.tensor.matmul(
                        pds[idx // 2][:, 512 * (idx % 2):512 * (idx % 2)
                                      + FCH].rearrange("p (r c) -> p r c",
                                                       c=56),
                        t["dwcdiag"][:, k, g, :],
                        vp[:, 8 * j + dy:8 * j + dy + 8, dx:dx + 56],
                        start=(k == 0), stop=(k == len(PE_TAPS) - 1))
            pas = [_bank(pools, "pa") for _ in half]
            for q in range(2):
                for idx, j in enumerate(half):
                    nc.tensor.matmul(
                        pas[idx][64 * q:64 * (q + 1), 0:FCH],
                        t["av_ext"][0:113, 2 * g + q, :],
                        t["w2T"][0:113, 2 * g + q, FCH * j:FCH * (j + 1)],
                        start=True, stop=True)
            for idx, j in enumerate(half):
                pd = pds[idx // 2][:, 512 * (idx % 2):512 * (idx % 2) + FCH]
                pa = pas[idx]
                upt = pools["upch"].tile([128, FCH], BD, name="up", tag="up")
                with nc.allow_low_precision("bf16 activations"):
                    # u' = u * (1/s2)          (DVE, PSUM read)
                    nc.vector.tensor_tensor(
                        out=upt[:, :], in0=pa[:, 0:FCH],
                        in1=t["rbc"][:, g, FCH * j:FCH * (j + 1)], op=OP.mult)
                    # u' += dve-tap            (DVE, SBUF FMA)
                    dy, dx = TAPS[DVE_TAP]
                    nc.vector.scalar_tensor_tensor(
                        out=upt[:, :].rearrange("p (r c) -> p r c", c=56),
                        in0=vp[:, 8 * j + dy:8 * j + dy + 8, dx:dx + 56],
                        scalar=t["dwcw"][:, g, DVE_TAP:DVE_TAP + 1],
                        in1=upt[:, :].rearrange("p (r c) -> p r c", c=56),
                        op0=OP.mult, op1=OP.add)
                    # sum = (dwc_pe + dwc_b) + u'   (DVE, PSUM read)
                    nc.vector.scalar_tensor_tensor(
                        out=t["sum"][:, g, FCH * j:FCH * (j + 1)],
                        in0=pd, scalar=t["dwcb"][:, g, :],
                        in1=upt[:, :], op0=OP.add, op1=OP.add)

    # ---------------- proj + bias + residual ----------------
    for mt in range(CT):
        for js in _grp(NF, 2):
            j0, nj = js[0], len(js)
            pp = _big(pools, "pp")
            for kt in range(CT):
                for j in js:
                    nc.tensor.matmul(
                        pp[:, 512 * (j - j0):512 * (j - j0) + FCH],
                        t["wproj"][:, kt, 128 * mt:128 * (mt + 1)],
                        t["sum"][:, kt, FCH * j:FCH * (j + 1)],
                        start=(kt == 0), stop=(kt == CT - 1))
            src = (pp[:, 0:512 * nj].rearrange("p (j f) -> p j f", f=512)
                   [:, :, 0:FCH])
            ost = pools["ostp"].tile([128, 2, FCH], FD, name="ost",
                                     tag="ost", bufs=2)
            resid = (xbf[:, mt, FCH * j0:FCH * (j0 + nj)]
                     .rearrange("p (j f) -> p j f", f=FCH))
            nc.vector.scalar_tensor_tensor(
                out=ost[:, 0:nj, :], in0=src, scalar=t["projb"][:, mt, :],
                in1=resid, op0=OP.add, op1=OP.add)
            nc.gpsimd.dma_start(
                out=d["out32"][b, 128 * mt:128 * (mt + 1),
                               FCH * j0:FCH * (j0 + nj)],
                in_=ost[:, 0:nj, :])


def host_prep(x, qkv_w, proj_w, proj_b, dwc_w, dwc_b):
    xf = np.ascontiguousarray(x.reshape(B, C, N), dtype=FP32)
    wqkvT = np.ascontiguousarray(np.asarray(qkv_w, FP32).T.astype(BF16))
    wprojT = np.ascontiguousarray(np.asarray(proj_w, FP32).T.astype(BF16))
    projb = np.ascontiguousarray(np.asarray(proj_b, FP32).reshape(C, 1))
    dwcb = np.ascontiguousarray(np.asarray(dwc_b, FP32).reshape(C, 1))
    w33 = np.asarray(dwc_w, FP32).reshape(C, 9)
    dd = np.zeros((len(PE_TAPS), CT, 128, 128), FP32)
    for k, ti in enumerate(PE_TAPS):
        for g in range(CT):
            np.fill_diagonal(dd[k, g], w33[128 * g:128 * (g + 1), ti])
    dwcdiag = np.ascontiguousarray(dd.astype(BF16))
    dwcw = np.ascontiguousarray(w33)
    maps = []
    for c in range(NCORES):
        xs = xf[BPC * c:BPC * (c + 1)]
        maps.append(dict(
            xbf=np.ascontiguousarray(xs.astype(BF16)),
            wqkvT=wqkvT, wprojT=wprojT, projb=projb,
            dwcdiag=dwcdiag, dwcw=dwcw, dwcb=dwcb,
        ))
    return maps


_NC_CACHE = {}


def kernel(x, qkv_w, proj_w, proj_b, dwc_w, dwc_b, trace=False):
    if "nc" not in _NC_CACHE:
        _NC_CACHE["nc"] = build_bass()
    nc = _NC_CACHE["nc"]
    maps = host_prep(np.asarray(x), np.asarray(qkv_w), np.asarray(proj_w),
                     np.asarray(proj_b), np.asarray(dwc_w), np.asarray(dwc_b))
    res = run_bass_kernel_spmd(nc, maps, core_ids=list(range(NCORES)),
                               trace=trace)
    outs = [r["out32"].reshape(BPC, C, HH, WW) for r in res.results]
    full = np.concatenate(outs, axis=0).astype(np.float32)
    if trace:
        return full, res
    return full

